# revision 35
# baseline (speedup 1.0000x reference)
import sys
sys.path.insert(0, "/opt/trn_rl_repo")
import numpy as np

import concourse.bass as bass
import concourse.bacc as bacc
import concourse.tile as tile
import concourse.mybir as mybir
from concourse.bass_utils import run_bass_kernel_spmd
from concourse.masks import make_identity
import ml_dtypes

USE_DR = True   # fp8 DoubleRow correction passes
NPF8 = ml_dtypes.float8_e4m3fn
F8 = mybir.dt.float8e4
DRS = 256.0     # 2^8 scale for fp8 correction operands

F32 = mybir.dt.float32
F16 = mybir.dt.float16
U32 = mybir.dt.uint32
AFT = mybir.ActivationFunctionType

ALPHA = 0.25
Q = 4          # quantizers
K = 1024       # codebook size
D = 512        # dim
T = 2048       # tokens per batch element
B = 8          # batch -> one per core
P = 128
DC = D // P    # 4 d-chunks
NT = T // P    # 16 token tiles
KB = K // 512  # 2 k-blocks for matmul moving width


def _build_program():
    nc = bacc.Bacc("TRN2", target_bir_lowering=False, debug=False, num_devices=B)

    x_ext = nc.declare_dram_parameter("x", [D, T], F32, isOutput=False)
    cbth_ext = nc.declare_dram_parameter("cbth", [Q, D, K], F16, isOutput=False)
    cbtl_ext = nc.declare_dram_parameter("cbtl", [Q, D, K], F16, isOutput=False)
    if USE_DR:
        cbdr_ext = nc.declare_dram_parameter("cbdr", [Q, DC, P, 2, K], F8, isOutput=False)
    negc2_ext = nc.declare_dram_parameter("negc2", [Q, P, K], F32, isOutput=False)
    cbr_ext = [
        nc.declare_dram_parameter(f"cbr{q}", [K, D], F32, isOutput=False)
        for q in range(Q)
    ]
    zq_ext = nc.declare_dram_parameter("zq", [D, T], F32, isOutput=True)

    with tile.TileContext(nc) as tc:
        with (
            tc.tile_pool(name="const", bufs=1) as const_pool,
            tc.tile_pool(name="data", bufs=1) as data_pool,
            tc.tile_pool(name="work", bufs=3) as work_pool,
            tc.tile_pool(name="work2", bufs=2) as work2_pool,
            tc.tile_pool(name="psum_s", bufs=3, space="PSUM") as psum_s_pool,
            tc.tile_pool(name="psum_c", bufs=3, space="PSUM") as psum_c_pool,
            tc.tile_pool(name="psum_q", bufs=2, space="PSUM") as psum_q_pool,
        ):
            ident = const_pool.tile([P, P], F32)
            make_identity(nc, ident[:])

            # persistent SBUF data, [d,t] layout: 3D [128, dc, T]
            x_sb = data_pool.tile([P, DC, T], F32)
            r_sb = data_pool.tile([P, DC, T], F32)
            rh_sb = data_pool.tile([P, DC, T], F16)
            if not USE_DR:
                rl_sb = data_pool.tile([P, DC, T], F16)
            cbth_sb = data_pool.tile([P, Q, DC, K], F16)
            if USE_DR:
                cbdr_sb = data_pool.tile([P, Q, DC, 2, K], F8)
                r8l8_sb = data_pool.tile([P, 2, DC, T], F8)
            else:
                cbtl_sb = data_pool.tile([P, Q, DC, K], F16)
            negc2_sb = data_pool.tile([P, Q, K], F32)
            maxv8 = data_pool.tile([P, 8], F32)
            nc.vector.memset(maxv8[:, 1:8], -3.0e38)

            TQ = T // 4
            for tb in range(4):
                for dc in range(DC):
                    nc.sync.dma_start(x_sb[:, dc, tb * TQ:(tb + 1) * TQ],
                                      x_ext[dc * P:(dc + 1) * P, tb * TQ:(tb + 1) * TQ])
                if tb == 0:
                    for dc in range(DC):
                        nc.sync.dma_start(cbth_sb[:, 0, dc], cbth_ext[0, dc * P:(dc + 1) * P, :])
                        if USE_DR:
                            nc.sync.dma_start(cbdr_sb[:, 0, dc], cbdr_ext[0, dc])
                        else:
                            nc.sync.dma_start(cbtl_sb[:, 0, dc], cbtl_ext[0, dc * P:(dc + 1) * P, :])
                    nc.sync.dma_start(negc2_sb[:, 0], negc2_ext[0])
            for q in range(1, Q):
                for dc in range(DC):
                    nc.sync.dma_start(cbth_sb[:, q, dc], cbth_ext[q, dc * P:(dc + 1) * P, :])
                    if USE_DR:
                        nc.sync.dma_start(cbdr_sb[:, q, dc], cbdr_ext[q, dc])
                    else:
                        nc.sync.dma_start(cbtl_sb[:, q, dc], cbtl_ext[q, dc * P:(dc + 1) * P, :])
                nc.sync.dma_start(negc2_sb[:, q], negc2_ext[q])

            # initial splits of r = x, per token tile
            for ti in range(NT):
                tsl = slice(ti * P, (ti + 1) * P)
                nc.scalar.activation(rh_sb[:, :, tsl], x_sb[:, :, tsl], AFT.Identity)
                if USE_DR:
                    rl_t = work_pool.tile([P, DC, P], F16, tag="rl")
                    nc.gpsimd.tensor_sub(rl_t[:], x_sb[:, :, tsl], rh_sb[:, :, tsl])
                    nc.scalar.activation(r8l8_sb[:, 0, :, tsl], x_sb[:, :, tsl], AFT.Identity)
                    nc.scalar.activation(r8l8_sb[:, 1, :, tsl], rl_t[:], AFT.Identity, scale=DRS)
                else:
                    nc.vector.tensor_sub(rl_sb[:, :, tsl], x_sb[:, :, tsl], rh_sb[:, :, tsl])

            for q in range(Q):
                first = q == 0
                last = q == Q - 1
                for ti in range(NT):
                    tsl = slice(ti * P, (ti + 1) * P)
                    # dots: psum[t,k] += 2*r.c
                    score_sb = work_pool.tile([P, K], F32, tag="score")
                    if USE_DR:
                        # main pass fp16 h.H; corrections h.L + l.H via one fp8
                        # DoubleRow pass at 2^8 scale in a separate psum
                        for kb in range(KB):
                            ksl = slice(kb * 512, (kb + 1) * 512)
                            psum_s = psum_s_pool.tile([P, 512], F32, tag="psum_s")
                            psum_c = psum_c_pool.tile([P, 512], F32, tag="psum_c")
                            for dc in range(DC):
                                nc.tensor.matmul(
                                    out=psum_s[:],
                                    lhsT=rh_sb[:, dc, tsl],
                                    rhs=cbth_sb[:, q, dc, ksl],
                                    start=(dc == 0), stop=(dc == DC - 1),
                                )
                            for dc in range(DC):
                                nc.tensor.matmul(
                                    out=psum_c[:],
                                    lhsT=r8l8_sb[:, :, dc, tsl],
                                    rhs=cbdr_sb[:, q, dc, :, ksl],
                                    start=(dc == 0), stop=(dc == DC - 1),
                                    perf_mode=mybir.MatmulPerfMode.DoubleRow,
                                )
                            corr = work2_pool.tile([P, 512], F32, tag="corr")
                            nc.scalar.activation(corr[:], psum_c[:], AFT.Identity, scale=1.0 / DRS)
                            tmp = work2_pool.tile([P, 512], F32, tag="tmpc2")
                            nc.gpsimd.tensor_add(tmp[:], corr[:], negc2_sb[:, q, ksl])
                            nc.vector.tensor_add(score_sb[:, ksl], psum_s[:], tmp[:])
                    else:
                        psum_s = psum_s_pool.tile([P, K], F32, tag="psum_s")
                        for kb in range(KB):
                            ksl = slice(kb * 512, (kb + 1) * 512)
                            n_mm = 0
                            for dc in range(DC):
                                for (lhs, rhs) in (
                                    (rh_sb, cbth_sb), (rh_sb, cbtl_sb), (rl_sb, cbth_sb),
                                ):
                                    nc.tensor.matmul(
                                        out=psum_s[:, ksl],
                                        lhsT=lhs[:, dc, tsl],
                                        rhs=rhs[:, q, dc, ksl],
                                        start=(n_mm == 0), stop=(n_mm == 3 * DC - 1),
                                    )
                                    n_mm += 1
                        nc.vector.tensor_add(score_sb[:], psum_s[:], negc2_sb[:, q])
                    max8 = work_pool.tile([P, 8], F32, tag="max8")
                    idx8 = work_pool.tile([P, 8], U32, tag="idx8")
                    nc.vector.max(out=max8[:], in_=score_sb[:])
                    nc.vector.max_index(out=idx8[:], in_max=max8[:], in_values=score_sb[:])

                    # gather exact f32 codebook rows -> [t, d]
                    qrow = work_pool.tile([P, D], F32, tag="qrow")
                    nc.gpsimd.indirect_dma_start(
                        out=qrow[:],
                        out_offset=None,
                        in_=cbr_ext[q][:],
                        in_offset=bass.IndirectOffsetOnAxis(ap=idx8[:, 0:1], axis=0),
                    )

                    # transpose back to [d, t] and update residual
                    psum_q = psum_q_pool.tile([P, DC, P], F32, tag="psum_q")
                    for dc in range(DC):
                        nc.tensor.transpose(
                            out=psum_q[:, dc],
                            in_=qrow[:, dc * P:(dc + 1) * P],
                            identity=ident[:],
                        )
                    src = x_sb if first else r_sb
                    nc.vector.tensor_sub(r_sb[:, :, tsl], src[:, :, tsl], psum_q[:])
                    if not last:
                        nc.scalar.activation(rh_sb[:, :, tsl], r_sb[:, :, tsl], AFT.Identity)
                        if USE_DR:
                            rl_t = work_pool.tile([P, DC, P], F16, tag="rl")
                            nc.gpsimd.tensor_sub(rl_t[:], r_sb[:, :, tsl], rh_sb[:, :, tsl])
                            nc.scalar.activation(r8l8_sb[:, 0, :, tsl], r_sb[:, :, tsl], AFT.Identity)
                            nc.scalar.activation(r8l8_sb[:, 1, :, tsl], rl_t[:], AFT.Identity, scale=DRS)
                        else:
                            nc.vector.tensor_sub(rl_sb[:, :, tsl], r_sb[:, :, tsl], rh_sb[:, :, tsl])
                    else:
                        # z_q = x - r4, overwrite x; stream out per tile
                        nc.vector.tensor_sub(x_sb[:, :, tsl], x_sb[:, :, tsl], r_sb[:, :, tsl])
                        for dc in range(DC):
                            nc.sync.dma_start(zq_ext[dc * P:(dc + 1) * P, tsl], x_sb[:, dc, tsl])

    nc.compile()
    return nc


_CACHE = {}


def _get_program():
    if "nc" not in _CACHE:
        _CACHE["nc"] = _build_program()
    return _CACHE["nc"]


def _prep_shared(codebooks):
    cb = np.asarray(codebooks, np.float32)               # [Q, K, D]
    cbt2 = 2.0 * np.transpose(cb, (0, 2, 1))             # [Q, D, K]
    cbth = cbt2.astype(np.float16)
    cbtl = (cbt2 - cbth.astype(np.float32)).astype(np.float16)
    negc2 = -np.sum(cb.astype(np.float64) * cb, axis=2).astype(np.float32)  # [Q, K]
    negc2_b = np.broadcast_to(negc2[:, None, :], (Q, P, K)).copy()
    shared = {"cbth": cbth, "cbtl": cbtl, "negc2": negc2_b}
    if USE_DR:
        cbtl32 = cbt2 - cbth.astype(np.float32)          # [Q, D, K] fp16-lo residual
        cbdr = np.empty((Q, DC, P, 2, K), NPF8)
        for q in range(Q):
            for dc in range(DC):
                cbdr[q, dc, :, 0, :] = (256.0 * cbtl32[q, dc * P:(dc + 1) * P]).astype(NPF8)
                cbdr[q, dc, :, 1, :] = cbt2[q, dc * P:(dc + 1) * P].astype(NPF8)
        shared["cbdr"] = cbdr
    for q in range(Q):
        shared[f"cbr{q}"] = np.ascontiguousarray(cb[q])
    return shared


def kernel(z_e, codebooks, _timing=None):
    z_e = np.asarray(z_e, np.float32)
    nc = _get_program()
    shared = _prep_shared(codebooks)
    in_maps = [dict(shared, x=np.ascontiguousarray(z_e[i])) for i in range(B)]
    res = run_bass_kernel_spmd(nc, in_maps, list(range(B)),
                               **({} if _timing is None else _timing))
    if _timing is not None:
        _CACHE["last_results"] = res
    zq = np.stack([res.results[i]["zq"] for i in range(B)], axis=0)
    r4 = z_e - zq
    loss = np.float32(ALPHA * np.mean(np.square(r4), dtype=np.float64))
    return (zq, loss, np.float32(0.0))
